# revision 4
# baseline (speedup 1.0000x reference)
"""Trainium2 Bass kernel for nn_Combineall (ragged graph readout + BN bilinear + conv similarity).

Strategy (8 NeuronCores, data-parallel over graphs, interleaved for load balance):
  phase 1: stream fp32 shards from HBM, cast to a bf16 SBUF cache, ACT-square,
           PE matmuls with one-hot selectors -> per-graph sums S and global sum(x^2).
  boundary: on-device BN stats + AllReduce([128,8]) across the 8 cores,
           tg = tanh(mean @ W) expanded to a per-chunk "tg pair wall" via a SEL matmul.
  phase 2: batched SBUF->SBUF DMA transposes of the bf16 cache (feature-major layout),
           ACT-fused t = tanh(g*x + b2) (per-partition scale/bias),
           scoreh window sums via fused scalar_tensor_tensor accum,
           per-node gate dots via PE (x_T stationary, tg-pair moving) + mask select,
           gated readout e via weighted one-hot PE matmuls.
  host:    sharding/index prep, window boundary corrections, BN pad terms,
           and the tiny VectorSimilarity convolutions.
"""
import sys
import numpy as np

sys.path.insert(0, "/opt/trn_rl_repo")

N_CORES = 8
F = 256
EPS = 1e-5
BP1 = 4            # node-tiles per phase-1 DMA batch
BP2 = 8            # node-tiles per phase-2 batch (1024 nodes)
STRIP = 32         # chunks (node tiles) per d-strip
WIN = 256          # scoreh window size in nodes

_CACHE = {}


# ----------------------------------------------------------------------------
def _vector_similarity(e1, e2, ws):
    from numpy.lib.stride_tricks import sliding_window_view
    res = []
    for ki, wk in enumerate(ws):
        k = ki + 1
        for si in range(3):
            s = si + 1
            w = np.asarray(wk[si], np.float64)[:, 0, :]     # [4, k]
            win1 = sliding_window_view(np.asarray(e1, np.float64), k, axis=1)[:, ::s, :]
            win2 = sliding_window_view(np.asarray(e2, np.float64), k, axis=1)[:, ::s, :]
            c1 = np.einsum("blk,ok->bol", win1, w)
            c2 = np.einsum("blk,ok->bol", win2, w)
            ham = (np.tanh(c1) * np.tanh(c2)).mean(axis=(1, 2))
            cos = np.exp(-np.square(c1 - c2).sum(axis=-1) / 4.0).mean(axis=-1)
            res.append(np.stack([ham, cos], axis=-1))
    return res


def _numpy_reference(x1, x2, W_read, gamma, beta, ws, batch1, batch2, B, nmax):
    def readout(x, batch):
        cnt = np.bincount(batch, minlength=B).astype(np.float64)
        S = np.zeros((B, x.shape[1]))
        np.add.at(S, batch, x.astype(np.float64))
        mean = S / np.maximum(cnt, 1)[:, None]
        tg = np.tanh(mean @ np.asarray(W_read, np.float64))
        coefs = 1.0 / (1.0 + np.exp(-(x.astype(np.float64) * tg[batch]).sum(1)))
        e = np.zeros((B, x.shape[1]))
        np.add.at(e, batch, coefs[:, None] * x.astype(np.float64))
        return e

    e1 = readout(x1, batch1)
    e2 = readout(x2, batch2)
    T = B * nmax

    def bn_tanh(x):
        S = x.astype(np.float64).sum(0)
        Q = (x.astype(np.float64) ** 2).sum(0)
        m = S / T
        v = Q / T - m * m
        g = np.asarray(gamma, np.float64) / np.sqrt(v + EPS)
        b2 = np.asarray(beta, np.float64) - m * g
        return np.tanh(x.astype(np.float64) * g + b2), np.tanh(b2)

    t1, c1 = bn_tanh(x1)
    t2, c2 = bn_tanh(x2)
    cnt1 = np.bincount(batch1, minlength=B)
    scoreh = np.zeros((B, x1.shape[1]))
    np.add.at(scoreh, batch1, t1 * t2)
    scoreh += (nmax - cnt1)[:, None] * (c1 * c2)[None, :]
    res = _vector_similarity(e1, e2, ws)
    return np.concatenate(res + [scoreh], axis=-1).astype(np.float32)


# ----------------------------------------------------------------------------
class _Meta:
    pass


def _plan(counts, B):
    starts = np.zeros(B + 1, np.int64)
    starts[1:] = np.cumsum(counts)
    metas = []
    for c in range(N_CORES):
        m = _Meta()
        m.graphs = np.arange(c, B, N_CORES)
        m.cnt = counts[m.graphs]
        m.gstart = starts[m.graphs]
        m.n = int(m.cnt.sum())
        m.loc = np.zeros(len(m.graphs) + 1, np.int64)
        m.loc[1:] = np.cumsum(m.cnt)
        metas.append(m)
    NT = max((m.n + 127) // 128 for m in metas)
    NT = ((NT + BP2 - 1) // BP2) * BP2          # multiple of phase-2 batch
    for m in metas:
        m.npad = NT * 128
        gl = np.full(m.npad, -1, np.int64)
        for j in range(len(m.graphs)):
            gl[m.loc[j]:m.loc[j + 1]] = j
        m.gl = gl
    return metas, NT


def _core_inputs(m, NT, x1, x2, W_read, gamma, beta):
    import ml_dtypes
    bf16 = ml_dtypes.bfloat16
    NSTR = (NT + STRIP - 1) // STRIP
    NG = len(m.graphs)
    gl = m.gl

    def shard(x):
        out = np.zeros((m.npad, F), np.float32)
        pos = 0
        for j in range(NG):
            a, b = m.gstart[j], m.gstart[j] + m.cnt[j]
            out[pos:pos + m.cnt[j]] = x[a:b]
            pos += m.cnt[j]
        return out

    onehot = np.zeros((m.npad, 64), np.float32)
    valid = gl >= 0
    onehot[np.arange(m.npad)[valid], gl[valid]] = 1.0

    ga = np.zeros(NT, np.int64)
    mask = np.zeros((NSTR, 128, 2 * STRIP), np.float32)
    for t in range(NT):
        g0 = gl[t * 128]
        ga[t] = min(int(g0), 62) if g0 >= 0 else 62
        s, ci = divmod(t, STRIP)
        seg = gl[t * 128:(t + 1) * 128]
        d = seg - ga[t]
        p = np.arange(128)
        ok0 = (d == 0)
        ok1 = (d == 1)
        mask[s, p[ok0], 2 * ci] = 1.0
        mask[s, p[ok1], 2 * ci + 1] = 1.0

    sel = np.zeros((64, 2 * NT), np.float32)
    for t in range(NT):
        sel[ga[t], 2 * t] = 1.0
        if ga[t] + 1 < 64:
            sel[ga[t] + 1, 2 * t + 1] = 1.0

    invcnt = np.zeros((128, 2), np.float32)
    invcnt[:NG, 0] = 1.0 / m.cnt
    invcnt[:, 1] = 1.0

    wre = np.asarray(W_read, np.float32).reshape(2, 128, F).transpose(1, 0, 2)
    gbv = np.stack([gamma.reshape(2, 128)[0], gamma.reshape(2, 128)[1],
                    beta.reshape(2, 128)[0], beta.reshape(2, 128)[1]], axis=1)

    return {
        "x1": shard(x1), "x2": shard(x2),
        "onehot": onehot.reshape(NT, 128, 64).astype(bf16),
        "mask": mask,
        "sel": sel.astype(bf16),
        "invcnt": invcnt,
        "w_read": np.ascontiguousarray(wre).astype(bf16),
        "gammabeta": np.ascontiguousarray(gbv).astype(np.float32),
        "onescol": np.ones((128, 1), np.float32).astype(bf16),
    }


# ----------------------------------------------------------------------------
def _build(NT, T_bn):
    from concourse import bacc, tile, mybir

    F32, BF16 = mybir.dt.float32, mybir.dt.bfloat16
    AF = mybir.ActivationFunctionType
    ALU = mybir.AluOpType

    NSTR = (NT + STRIP - 1) // STRIP
    NB1 = NT // BP1
    NB2 = NT // BP2
    NW = NT * 128 // WIN

    nc = bacc.Bacc("TRN2", target_bir_lowering=False, debug=False, num_devices=N_CORES)

    x_in = [nc.dram_tensor(n, [NT * 128, F], F32, kind="ExternalInput").ap()
            for n in ("x1", "x2")]
    oh_in = nc.dram_tensor("onehot", [NT, 128, 64], BF16, kind="ExternalInput").ap()
    mk_in = nc.dram_tensor("mask", [NSTR, 128, 2 * STRIP], F32, kind="ExternalInput").ap()
    sel_in = nc.dram_tensor("sel", [64, 2 * NT], BF16, kind="ExternalInput").ap()
    ic_in = nc.dram_tensor("invcnt", [128, 2], F32, kind="ExternalInput").ap()
    w_in = nc.dram_tensor("w_read", [128, 2, F], BF16, kind="ExternalInput").ap()
    gb_in = nc.dram_tensor("gammabeta", [128, 4], F32, kind="ExternalInput").ap()
    on_in = nc.dram_tensor("onescol", [128, 1], BF16, kind="ExternalInput").ap()

    s_out = [nc.dram_tensor(n, [64, F], F32, kind="ExternalOutput").ap()
             for n in ("s1_rows", "s2_rows")]
    e_out = [nc.dram_tensor(n, [64, F], F32, kind="ExternalOutput").ap()
             for n in ("e1_part", "e2_part")]
    sh_out = nc.dram_tensor("sh_part", [128, 2 * NW], F32, kind="ExternalOutput").ap()
    gs_out = nc.dram_tensor("gstats", [128, 8], F32, kind="ExternalOutput").ap()

    with tile.TileContext(nc) as tc:
        with tc.tile_pool(name="cache", bufs=1) as cpool, \
             tc.tile_pool(name="consts", bufs=1) as kpool, \
             tc.tile_pool(name="psA", bufs=1, space="PSUM") as psA, \
             tc.tile_pool(name="psB", bufs=1, space="PSUM") as psB, \
             tc.tile_pool(name="psC", bufs=2, space="PSUM") as psC, \
             tc.tile_pool(name="dram", bufs=2, space="DRAM") as dpool, \
             tc.tile_pool(name="wk", bufs=2) as wk, \
             tc.tile_pool(name="wk1", bufs=1) as wk1:

            onehot = kpool.tile([128, NT, 64], BF16, tag="onehot", name="onehot")
            nc.sync.dma_start(onehot[:], oh_in.rearrange("t p g -> p t g"))
            maskt = kpool.tile([128, NSTR, 2 * STRIP], F32, tag="mask", name="mask")
            nc.sync.dma_start(maskt[:], mk_in.rearrange("s p c -> p s c"))
            selt = kpool.tile([64, 2 * NT], BF16, tag="sel", name="sel")
            nc.sync.dma_start(selt[:], sel_in[:])
            invcnt = kpool.tile([128, 2], F32, tag="invcnt", name="invcnt")
            nc.sync.dma_start(invcnt[:], ic_in[:])
            wread = kpool.tile([128, 2, F], BF16, tag="wread", name="wread")
            nc.sync.dma_start(wread[:], w_in[:])
            gb = kpool.tile([128, 4], F32, tag="gb", name="gb")
            nc.sync.dma_start(gb[:], gb_in[:])
            ones128 = kpool.tile([128, 1], BF16, tag="ones", name="ones")
            nc.sync.dma_start(ones128[:], on_in[:])

            cache = [cpool.tile([128, NT * F], BF16, tag=f"cache{i}", name=f"cache{i}") for i in range(2)]
            S_ps = [psA.tile([64, F], F32, tag=f"S{i}", name=f"S{i}") for i in range(2)]
            Q_ps = [psA.tile([1, F], F32, tag=f"Q{i}", name=f"Q{i}") for i in range(2)]

            # ======== phase 1: load + cast + S ========
            for b in range(NB1):
                t0 = b * BP1
                for i in range(2):
                    stage = wk.tile([128, BP1, F], F32, tag="stage", name="stage")
                    src = x_in[i].rearrange("(t p) f -> p t f", p=128)
                    nc.sync.dma_start(stage[:], src[:, t0:t0 + BP1, :])
                    cs = cache[i][:, t0 * F:(t0 + BP1) * F]
                    nc.vector.tensor_copy(
                        cs.rearrange("p (t f) -> p t f", t=BP1), stage[:])
                    for tt in range(BP1):
                        t = t0 + tt
                        nc.tensor.matmul(S_ps[i][:], onehot[:, t, :],
                                         cache[i][:, t * F:(t + 1) * F],
                                         start=(t == 0), stop=(t == NT - 1))
            # squares + Q (bf16, from cache)
            for i in range(2):
                for b in range(NB2):
                    t0 = b * BP2
                    xsq = wk.tile([128, BP2 * F], BF16, tag="xsq", name="xsq")
                    nc.scalar.activation(xsq[:], cache[i][:, t0 * F:(t0 + BP2) * F],
                                         AF.Square)
                    for tt in range(BP2):
                        t = t0 + tt
                        nc.tensor.matmul(Q_ps[i][:], ones128[:],
                                         xsq[:, tt * F:(tt + 1) * F],
                                         start=(t == 0), stop=(t == NT - 1))

            # ======== boundary: stats + tg wall ========
            S_sb = [wk1.tile([64, F], F32, tag=f"Ssb{i}", name=f"Ssb{i}") for i in range(2)]
            mean_bf = [wk1.tile([64, F], BF16, tag=f"mbf{i}", name=f"mbf{i}") for i in range(2)]
            rows = wk1.tile([1, 8 * 128], F32, tag="rows", name="rows")

            for i in range(2):
                nc.vector.tensor_copy(S_sb[i][:], S_ps[i][:])
                nc.sync.dma_start(s_out[i][:], S_sb[i][:])
                nc.vector.tensor_scalar(mean_bf[i][:], S_sb[i][:],
                                        invcnt[0:64, 0:1], None, ALU.mult)
                nc.vector.tensor_copy(rows[:, (4 * i) * 128:(4 * i + 2) * 128], Q_ps[i][:])
                sbf = wk.tile([64, F], BF16, tag="sbf", name="sbf")
                nc.vector.tensor_copy(sbf[:], S_sb[i][:])
                stp = psC.tile([1, F], F32, tag="bnd", name="bnd")
                nc.tensor.matmul(stp[:], ones128[0:64, :], sbf[:], start=True, stop=True)
                nc.vector.tensor_copy(rows[:, (4 * i + 2) * 128:(4 * i + 4) * 128], stp[:])

            statc_ps = psC.tile([128, 8], F32, tag="bnd", name="bnd")
            for j in range(8):
                nc.tensor.transpose(statc_ps[:, j:j + 1], rows[:, j * 128:(j + 1) * 128],
                                    invcnt[0:1, 1:2])
            statc = wk1.tile([128, 8], F32, tag="statc", name="statc")
            nc.vector.tensor_copy(statc[:], statc_ps[:])

            ib = dpool.tile([128, 8], F32)
            ob = dpool.tile([128, 8], F32)
            nc.gpsimd.dma_start(ib[:], statc[:])
            nc.gpsimd.collective_compute(
                "AllReduce", ALU.add, replica_groups=[list(range(N_CORES))],
                ins=[ib.opt()], outs=[ob.opt()])
            gstat = wk1.tile([128, 8], F32, tag="gstat", name="gstat")
            nc.gpsimd.dma_start(gstat[:], ob[:])
            nc.sync.dma_start(gs_out[:], gstat[:])

            gcol = [wk1.tile([128, 2], F32, tag=f"gcol{i}", name=f"gcol{i}") for i in range(2)]
            b2col = [wk1.tile([128, 2], F32, tag=f"b2col{i}", name=f"b2col{i}") for i in range(2)]
            for i in range(2):
                Q = gstat[:, 4 * i:4 * i + 2]
                St = gstat[:, 4 * i + 2:4 * i + 4]
                mcol = wk.tile([128, 2], F32, tag="sm", name="sm", bufs=8)
                nc.vector.tensor_scalar(mcol[:], St, 1.0 / T_bn, None, ALU.mult)
                msq = wk.tile([128, 2], F32, tag="sm", name="sm", bufs=8)
                nc.vector.tensor_tensor(msq[:], mcol[:], mcol[:], ALU.mult)
                v = wk.tile([128, 2], F32, tag="sm", name="sm", bufs=8)
                nc.vector.scalar_tensor_tensor(v[:], Q, 1.0 / T_bn, msq[:],
                                               ALU.mult, ALU.subtract)
                nc.vector.tensor_scalar(v[:], v[:], EPS, None, ALU.add)
                sq = wk.tile([128, 2], F32, tag="sm", name="sm", bufs=8)
                nc.scalar.activation(sq[:], v[:], AF.Sqrt)
                r = wk.tile([128, 2], F32, tag="sm", name="sm", bufs=8)
                nc.vector.reciprocal(r[:], sq[:])
                for _ in range(2):
                    r2 = wk.tile([128, 2], F32, tag="sm", name="sm", bufs=8)
                    nc.vector.tensor_tensor(r2[:], r[:], r[:], ALU.mult)
                    nc.vector.tensor_tensor(r2[:], r2[:], v[:], ALU.mult)
                    nc.vector.tensor_scalar(r2[:], r2[:], -0.5, 1.5, ALU.mult, ALU.add)
                    nc.vector.tensor_tensor(r[:], r[:], r2[:], ALU.mult)
                nc.vector.tensor_tensor(gcol[i][:], gb[:, 0:2], r[:], ALU.mult)
                mg = wk.tile([128, 2], F32, tag="sm", name="sm", bufs=8)
                nc.vector.tensor_tensor(mg[:], mcol[:], gcol[i][:], ALU.mult)
                nc.vector.scalar_tensor_tensor(b2col[i][:], mg[:], -1.0, gb[:, 2:4],
                                               ALU.mult, ALU.add)

            # tg wall: [128, 2, 2*NT] per tensor = tanh(W.T @ (mean.T @ SEL))
            tgw = [wk1.tile([128, 2, 2 * NT], BF16, tag=f"tgw{i}", name=f"tgw{i}") for i in range(2)]
            for i in range(2):
                mwall = []
                for h in range(2):
                    mw_ps = psC.tile([128, 2 * NT], F32, tag="bnd", name="bnd")
                    nc.tensor.matmul(mw_ps[:], mean_bf[i][:, h * 128:(h + 1) * 128],
                                     selt[:], start=True, stop=True)
                    mw = wk.tile([128, 2 * NT], BF16, tag="mw", name="mw")
                    nc.vector.tensor_copy(mw[:], mw_ps[:])
                    mwall.append(mw)
                for hp in range(2):
                    tg_ps = psC.tile([128, 2 * NT], F32, tag="bnd", name="bnd")
                    for h in range(2):
                        nc.tensor.matmul(tg_ps[:], wread[:, h, hp * 128:(hp + 1) * 128],
                                         mwall[h][:], start=(h == 0), stop=(h == 1))
                    nc.scalar.activation(tgw[i][:, hp, :], tg_ps[:], AF.Tanh)

            # ======== phase 2 ========
            sh_acc = wk1.tile([128, 2 * NW], F32, tag="sh", name="sh")
            wwall = [wk1.tile([128, NSTR * STRIP], F32, tag=f"ww{i}", name=f"ww{i}") for i in range(2)]
            e_ps = [psB.tile([64, F], F32, tag=f"e{i}", name=f"e{i}") for i in range(2)]
            dstrips = {}

            for b in range(NB2):
                t0 = b * BP2
                th = {}
                for i in range(2):
                    xT = wk.tile([128, 2 * BP2, 128], BF16, tag="xT", name="xT")
                    nc.sync.dma_start_transpose(
                        xT[:], cache[i][:, t0 * F:(t0 + BP2) * F])
                    for h in range(2):
                        tt_ = wk.tile([128, BP2 * 128], BF16, tag=f"t{i}{h}", name=f"t{i}{h}", bufs=1)
                        nc.scalar.activation(
                            tt_.rearrange("p (t j) -> p t j", t=BP2),
                            xT[:, h::2, :], AF.Tanh,
                            bias=b2col[i][:, h:h + 1], scale=gcol[i][:, h:h + 1])
                        th[(i, h)] = tt_
                    # d matmuls for this tensor's chunks
                    for tt in range(BP2):
                        t = t0 + tt
                        sidx, cidx = divmod(t, STRIP)
                        if cidx == 0:
                            dstrips[(i, sidx)] = psC.tile([128, 2 * STRIP], F32,
                                                          tag="bnd", name="dstrip")
                        dstr = dstrips[(i, sidx)]
                        for h in range(2):
                            nc.tensor.matmul(
                                dstr[:, 2 * cidx:2 * cidx + 2],
                                xT[:, 2 * tt + h, :],
                                tgw[i][:, h, 2 * t:2 * t + 2],
                                start=(h == 0), stop=(h == 1))
                        if t == (sidx + 1) * STRIP - 1 or t == NT - 1:
                            nchunk = cidx + 1
                            msel = wk.tile([128, 2 * STRIP], F32, tag="msel", name="msel")
                            nc.vector.tensor_tensor(msel[:, :2 * nchunk],
                                                    dstr[:, :2 * nchunk],
                                                    maskt[:, sidx, :2 * nchunk], ALU.mult)
                            mv = msel.rearrange("p (c two) -> p c two", two=2)
                            nc.vector.tensor_tensor(
                                wwall[i][:, sidx * STRIP:sidx * STRIP + nchunk],
                                mv[:, :nchunk, 0], mv[:, :nchunk, 1], ALU.add)
                            nc.scalar.activation(
                                wwall[i][:, sidx * STRIP:sidx * STRIP + nchunk],
                                wwall[i][:, sidx * STRIP:sidx * STRIP + nchunk],
                                AF.Tanh, scale=0.5)
                # scoreh windows of this batch
                nwb = BP2 * 128 // WIN
                for wi in range(nwb):
                    w = b * nwb + wi
                    a = wi * WIN
                    for h in range(2):
                        junk = wk.tile([128, WIN], BF16, tag="junk", name="junk")
                        nc.vector.scalar_tensor_tensor(
                            junk[:], th[(0, h)][:, a:a + WIN], 1.0,
                            th[(1, h)][:, a:a + WIN],
                            mybir.AluOpType.mult, mybir.AluOpType.mult,
                            accum_out=sh_acc[:, 2 * w + h:2 * w + h + 1])

            # e matmuls
            for i in range(2):
                for t in range(NT):
                    sidx, cidx = divmod(t, STRIP)
                    csel = wk.tile([128, 64], BF16, tag="csel", name="csel")
                    nc.vector.tensor_scalar(
                        csel[:], onehot[:, t, :],
                        wwall[i][:, sidx * STRIP + cidx:sidx * STRIP + cidx + 1],
                        None, ALU.mult)
                    nc.tensor.matmul(e_ps[i][:], csel[:],
                                     cache[i][:, t * F:(t + 1) * F],
                                     start=(t == 0), stop=(t == NT - 1))
                ef = wk.tile([64, F], F32, tag="ef", name="ef")
                nc.vector.tensor_copy(ef[:], e_ps[i][:])
                nc.sync.dma_start(e_out[i][:], ef[:])

            nc.sync.dma_start(sh_out[:], sh_acc[:])

    nc.compile()
    return nc


# ----------------------------------------------------------------------------
def _run_device(metas, NT, T_bn, x1, x2, W_read, gamma, beta):
    import os
    from concourse import bass_utils
    from concourse.bass_interp import get_hw_module

    key = (NT, float(T_bn))
    if key not in _CACHE:
        nc = _build(NT, T_bn)
        nc.m = get_hw_module(nc.m)
        _CACHE[key] = nc
    nc = _CACHE[key]

    in_maps = [_core_inputs(m, NT, x1, x2, W_read, gamma, beta) for m in metas]
    kw = {}
    if os.environ.get("KERNEL_TRACE"):
        kw = dict(trace=True, tmpdir=os.environ.get("KERNEL_TRACE_DIR") or None)
    res = bass_utils.run_bass_kernel_spmd(nc, in_maps, core_ids=list(range(N_CORES)),
                                          **kw)
    if res.exec_time_ns is not None:
        print("HW exec time: %d ns" % res.exec_time_ns)
    return res


# ----------------------------------------------------------------------------
def kernel(x1, x2, W_read, gamma, beta, w1, w2, w3, w4, w5, w6, w7, w8,
           batch1, batch2, batch_size, max_num_nodes):
    x1 = np.asarray(x1, np.float32)
    x2 = np.asarray(x2, np.float32)
    W_read = np.asarray(W_read, np.float32)
    gamma = np.asarray(gamma, np.float32)
    beta = np.asarray(beta, np.float32)
    ws = [np.asarray(w, np.float32) for w in (w1, w2, w3, w4, w5, w6, w7, w8)]
    batch1 = np.asarray(batch1)
    batch2 = np.asarray(batch2)
    B = int(batch_size)
    nmax = int(max_num_nodes)

    ok = (np.array_equal(batch1, batch2)
          and B % N_CORES == 0
          and x1.shape == x2.shape and x1.shape[1] == F
          and np.all(np.diff(batch1) >= 0))
    counts = np.bincount(batch1, minlength=B).astype(np.int64)
    ok = ok and counts.min() >= 130   # >=2 tiles guarantee <=2 graphs per 128-chunk

    if not ok:
        return _numpy_reference(x1, x2, W_read, gamma, beta, ws, batch1, batch2, B, nmax)

    try:
        metas, NT = _plan(counts, B)
        T_bn = float(B * nmax)
        res = _run_device(metas, NT, T_bn, x1, x2, W_read, gamma, beta)
    except Exception as ex:                        # pragma: no cover
        print("kernel: device path failed (%r); numpy fallback" % (ex,), file=sys.stderr)
        return _numpy_reference(x1, x2, W_read, gamma, beta, ws, batch1, batch2, B, nmax)

    # ---- host assembly ----
    import ml_dtypes
    gs = res.results[0]["gstats"].astype(np.float64)    # [128, 8]
    NW = NT * 128 // WIN

    stats = {}
    for i in range(2):
        Q = np.concatenate([gs[:, 4 * i], gs[:, 4 * i + 1]])
        St = np.concatenate([gs[:, 4 * i + 2], gs[:, 4 * i + 3]])
        m = St / T_bn
        v = Q / T_bn - m * m
        g = gamma.astype(np.float64) / np.sqrt(v + EPS)
        b2 = beta.astype(np.float64) - m * g
        stats[i] = (g, b2, np.tanh(b2))

    e1 = np.zeros((B, F), np.float64)
    e2 = np.zeros((B, F), np.float64)
    scoreh = np.zeros((B, F), np.float64)

    bf16 = ml_dtypes.bfloat16
    for c, m in enumerate(metas):
        r = res.results[c]
        gl = m.gl
        NG = len(m.graphs)
        for i, (e_acc, xf) in enumerate(((e1, x1), (e2, x2))):
            S = r[f"s{i+1}_rows"].astype(np.float64)[:NG]
            ep = r[f"e{i+1}_part"].astype(np.float64)[:NG]
            e_acc[m.graphs] = 0.5 * ep + 0.5 * S

        # scoreh: window sums + corrections
        sh = r["sh_part"].astype(np.float64)            # [128, 2*NW]
        wsum = np.concatenate([sh[:, 0::2], sh[:, 1::2]], axis=0)  # [256, NW]
        credit = gl[np.arange(NW) * WIN]                # window -> credited local graph
        # windows fully padded (credit<0): drop
        for j in range(NG):
            wmask = credit == j
            scoreh[m.graphs[j]] += wsum[:, wmask].sum(axis=1)
        # corrections: nodes whose true graph != credited graph of their window
        node_credit = credit[np.arange(m.npad) // WIN]
        bad = (gl != node_credit)
        bad &= ~((gl < 0) & (node_credit < 0))
        if bad.any():
            idx = np.nonzero(bad)[0]
            # t1*t2 for these nodes (pads -> x=0)
            g1, b21, c1t = stats[0]
            g2, b22, c2t = stats[1]
            xx1 = np.zeros((len(idx), F), np.float64)
            xx2 = np.zeros((len(idx), F), np.float64)
            real = gl[idx] >= 0
            # map local node -> original row
            loc2orig = np.full(m.npad, -1, np.int64)
            pos = 0
            for j in range(NG):
                nloc = int(m.cnt[j])
                loc2orig[pos:pos + nloc] = np.arange(m.gstart[j], m.gstart[j] + nloc)
                pos += nloc
            orig = loc2orig[idx]
            xb1 = x1.astype(bf16).astype(np.float64)
            xb2 = x2.astype(bf16).astype(np.float64)
            xx1[real] = xb1[orig[real]]
            xx2[real] = xb2[orig[real]]
            t1v = np.tanh(xx1 * g1 + b21)
            t2v = np.tanh(xx2 * g2 + b22)
            prod = t1v * t2v
            for k, n in enumerate(idx):
                cg, tg_ = node_credit[n], gl[n]
                if cg >= 0:
                    scoreh[m.graphs[cg]] -= prod[k]
                if tg_ >= 0:
                    scoreh[m.graphs[tg_]] += prod[k]

    # BN pad terms
    scoreh += (nmax - counts)[:, None].astype(np.float64) * (stats[0][2] * stats[1][2])[None, :]

    res_sim = _vector_similarity(e1, e2, ws)
    out = np.concatenate(res_sim + [scoreh], axis=-1).astype(np.float32)
    return out

